# revision 9
# baseline (speedup 1.0000x reference)
"""Trainium2 Bass kernel for CausalSelfAttention (no causal mask in reference).

Problem shapes: x [B=2, T=2048, C=1024], H=16 heads, D=64 head dim.
  q/k/v = x @ W{q,k,v}.T ; att = softmax(q k^T / sqrt(D)) ; y = att v
  out = y @ Wp.T + bp

Sharding over 8 NeuronCores: 4 head-groups (4 heads = 256 dims each) x 2
batches.  Core (g, b) computes a partial output for x[b] restricted to head
group g; the host sums the 4 head-group partials per batch and adds bp.

Per-core device program (matmul operands bf16, fp32 PSUM accumulate):
  1. QT = (Wq_g*scale) @ x^T   [256, T]   (d on partitions, t on free axis)
     KT = Wk_g @ x^T           [256, T]
     V  = x @ Wv_g^T           [T, 256]   (natural layout, + ones columns)
  2. per head h, per 512-wide t-chunk:
       S_T[s, t] = KT_h-tile @ QT_h       (scores transposed: s on partitions;
                                           two heads packed in PE row groups)
       P = exp(S_T)                       (no max subtraction: scores are O(1)
                                           by construction, exp is safe)
       Yaug^T = [V_h | 1...1]^T @ P       -> rows 0..63 unnormalized Y^T,
                                             rows 64..127 = softmax denom
                                             (replicated by the ones columns)
       Y^T = Yaug^T[0:64] * recip(rows 64..127)
  3. out_partial = Y^T-tiles^T @ Wp_g^T   [T, 1024]
All layouts chain with zero on-chip transposes.  Emission order interleaves
phase 1 with attention so the Scalar engine (exp, the throughput floor)
starts early and never starves.
"""

import numpy as np
import ml_dtypes

import concourse.bass as bass
import concourse.tile as tile
from concourse import mybir
from concourse.bacc import Bacc
from concourse.bass_utils import run_bass_kernel_spmd

BF16 = mybir.dt.bfloat16
F32 = mybir.dt.float32
NP_BF16 = ml_dtypes.bfloat16

P = 128
C = 1024
H = 16
D = 64
N_CORES = 8
N_GROUPS = 4              # head groups (tensor parallel)
N_BATCH = 2               # data parallel over B
HL = H // N_GROUPS        # 4 local heads
DL = HL * D               # 256 local head dims
CHUNK = 512               # t-chunk width (one PSUM bank of fp32)


def build_program(T: int = 2048) -> bass.Bass:
    KO = C // P            # k-tiles over the C contraction
    TT = T // P            # s/t tiles of 128
    NCH = T // CHUNK       # t-chunks
    KP = DL // P           # k-tiles over local head dims (2)

    nc = Bacc()
    xT_d = nc.declare_dram_parameter("xT", [C, T], BF16, isOutput=False)
    wqT_d = nc.declare_dram_parameter("wqT", [C, DL], BF16, isOutput=False)
    wkT_d = nc.declare_dram_parameter("wkT", [C, DL], BF16, isOutput=False)
    wvT_d = nc.declare_dram_parameter("wvT", [C, DL], BF16, isOutput=False)
    wpT_d = nc.declare_dram_parameter("wpT", [DL, C], BF16, isOutput=False)
    out_d = nc.declare_dram_parameter("out", [T, C], F32, isOutput=True)

    EXP = mybir.ActivationFunctionType.Exp

    with tile.TileContext(nc) as tc:
        with (
            tc.tile_pool(name="const", bufs=1) as cp,
            tc.tile_pool(name="p1ps", bufs=2, space="PSUM") as p1,
            tc.tile_pool(name="att_s", bufs=2, space="PSUM") as att_s,
            tc.tile_pool(name="accps", bufs=2, space="PSUM") as accps,
            tc.tile_pool(name="expp", bufs=34) as exp_pool,
            tc.tile_pool(name="normp", bufs=3) as norm_pool,
            tc.tile_pool(name="outp", bufs=3) as out_pool,
        ):
            xT_sb = cp.tile([P, KO, T], BF16)
            wqT_sb = cp.tile([P, KO, DL], BF16)
            wkT_sb = cp.tile([P, KO, DL], BF16)
            wvT_sb = cp.tile([P, KO, DL], BF16)
            wpT_sb = cp.tile([P, KP, C], BF16)
            QT_sb = cp.tile([P, KP, T], BF16)
            KT_sb = cp.tile([P, KP, T], BF16)
            # per head: 64 V columns then 64 ones columns; the ones columns
            # make the PV matmul emit the softmax denominator replicated
            # across PSUM partitions 64..127 (partition broadcast for free).
            Vaug_sb = cp.tile([P, TT, HL * 2 * D], BF16)
            YT_sb = cp.tile([P, KP, T], BF16)

            for w_d, w_sb in ((wkT_d, wkT_sb), (wqT_d, wqT_sb), (wvT_d, wvT_sb)):
                nc.sync.dma_start(
                    out=w_sb[:, :, :],
                    in_=w_d[:, :].rearrange("(ko p) d -> p ko d", p=P),
                )
            xT_r = xT_d[:, :].rearrange("(ko p) t -> ko p t", p=P)
            for k in range(KO):
                nc.sync.dma_start(out=xT_sb[:, k, :], in_=xT_r[k])
            nc.sync.dma_start(
                out=wpT_sb[:, :, :],
                in_=wpT_d[:, :].rearrange("(kp p) n -> p kp n", p=P),
            )

            vview = Vaug_sb.rearrange("p tt (h e) -> p tt h e", e=2 * D)
            nc.vector.memset(vview[:, :, :, D : 2 * D], 1.0)

            # ---------- emission helpers ----------
            def emit_qk_group(w_sb, o_sb, m, ch):
                ps = p1.tile([P, CHUNK], F32, tag="ps", name="ps")
                for k in range(KO):
                    nc.tensor.matmul(
                        ps,
                        lhsT=w_sb[:, k, m * P : (m + 1) * P],
                        rhs=xT_sb[:, k, ch * CHUNK : (ch + 1) * CHUNK],
                        start=(k == 0),
                        stop=(k == KO - 1),
                    )
                nc.vector.tensor_copy(
                    out=o_sb[:, m, ch * CHUNK : (ch + 1) * CHUNK], in_=ps
                )

            def emit_v_group(m):
                ps = p1.tile([P, CHUNK], F32, tag="ps", name="ps")
                for k in range(KO):
                    nc.tensor.matmul(
                        ps[:, 0:DL],
                        lhsT=xT_sb[:, k, m * P : (m + 1) * P],
                        rhs=wvT_sb[:, k, :],
                        start=(k == 0),
                        stop=(k == KO - 1),
                    )
                nc.vector.tensor_copy(
                    out=vview[:, m, :, 0:D],
                    in_=ps[:, 0:DL].rearrange("p (h e) -> p h e", e=D),
                )

            exps = {}  # (ch, hp) -> list of exp tiles

            def emit_sexp(ch, hp):
                t0 = ch * CHUNK
                lst = []
                for s in range(TT):
                    ps_s = att_s.tile([P, 2 * CHUNK], F32, tag="s", name="ps_s")
                    # two heads packed into PE row groups (K=64 each)
                    nc.tensor.matmul(
                        ps_s[:, 0:CHUNK],
                        lhsT=KT_sb[0:64, hp, s * P : (s + 1) * P],
                        rhs=QT_sb[0:64, hp, t0 : t0 + CHUNK],
                        start=True,
                        stop=True,
                    )
                    nc.tensor.matmul(
                        ps_s[:, CHUNK : 2 * CHUNK],
                        lhsT=KT_sb[64:128, hp, s * P : (s + 1) * P],
                        rhs=QT_sb[64:128, hp, t0 : t0 + CHUNK],
                        start=True,
                        stop=True,
                    )
                    ex = exp_pool.tile([P, 2 * CHUNK], BF16, tag="e", name="ex")
                    nc.scalar.activation(out=ex, in_=ps_s, func=EXP)
                    lst.append(ex)
                exps[(ch, hp)] = lst

            def emit_pv(ch, hp):
                t0 = ch * CHUNK
                lst = exps.pop((ch, hp))
                ps_y = {}
                for ha in range(2):
                    ps_y[ha] = accps.tile([P, CHUNK], F32, tag="acc", name="ps_y")
                for s in range(TT):
                    for ha in range(2):
                        h = hp * 2 + ha
                        nc.tensor.matmul(
                            ps_y[ha],
                            lhsT=Vaug_sb[:, s, h * 2 * D : (h + 1) * 2 * D],
                            rhs=lst[s][:, ha * CHUNK : (ha + 1) * CHUNK],
                            start=(s == 0),
                            stop=(s == TT - 1),
                        )
                for ha in range(2):
                    recip = norm_pool.tile([D, CHUNK], F32, tag="r", name="recip")
                    nc.vector.reciprocal(out=recip, in_=ps_y[ha][D : 2 * D, :])
                    nc.vector.tensor_mul(
                        out=YT_sb[ha * D : (ha + 1) * D, hp, t0 : t0 + CHUNK],
                        in0=ps_y[ha][0:D, :],
                        in1=recip,
                    )

            def emit_outproj(ch):
                for mt in range(CHUNK // P):
                    m = ch * (CHUNK // P) + mt
                    for n2 in range(C // CHUNK):
                        ps_o = accps.tile([P, CHUNK], F32, tag="acc", name="ps_o")
                        for kk in range(KP):
                            nc.tensor.matmul(
                                ps_o,
                                lhsT=YT_sb[:, kk, m * P : (m + 1) * P],
                                rhs=wpT_sb[:, kk, n2 * CHUNK : (n2 + 1) * CHUNK],
                                start=(kk == 0),
                                stop=(kk == KP - 1),
                            )
                        o_sb = out_pool.tile([P, CHUNK], F32, tag="o", name="o_sb")
                        nc.vector.tensor_copy(out=o_sb, in_=ps_o)
                        nc.sync.dma_start(
                            out=out_d[
                                m * P : (m + 1) * P,
                                n2 * CHUNK : (n2 + 1) * CHUNK,
                            ],
                            in_=o_sb,
                        )

            # ---------- emission order ----------
            # K fully, then Q chunk 0, then chunk-0 scores+exp so the Scalar
            # engine (the throughput floor) starts ~15us in; V and the
            # remaining Q chunks fill PE time under those exps.
            for m in range(KP):
                for ch in range(NCH):
                    emit_qk_group(wkT_sb, KT_sb, m, ch)
            for m in range(KP):
                emit_qk_group(wqT_sb, QT_sb, m, 0)
            emit_sexp(0, 0)
            emit_sexp(0, 1)
            for m in range(TT):
                emit_v_group(m)
            for ch in range(1, NCH):
                for m in range(KP):
                    emit_qk_group(wqT_sb, QT_sb, m, ch)
            if NCH > 1:
                emit_sexp(1, 0)
                emit_sexp(1, 1)
            for ch in range(NCH):
                if ch >= 2:
                    emit_sexp(ch, 0)
                    emit_sexp(ch, 1)
                emit_pv(ch, 0)
                emit_pv(ch, 1)
                emit_outproj(ch)
    # run the Bacc passes (matmul-wait relocation, wait splitting, reg alloc)
    nc.finalize()
    return nc


def shard_inputs(x, Wk, Wq, Wv, Wp, T=2048):
    """Build the 8 per-core input dicts (host-side transposes + bf16 casts)."""
    scale = 1.0 / np.sqrt(np.float32(D))
    x = np.asarray(x, np.float32)
    Wk = np.asarray(Wk, np.float32)
    Wq = np.asarray(Wq, np.float32)
    Wv = np.asarray(Wv, np.float32)
    Wp = np.asarray(Wp, np.float32)

    xT = [
        np.ascontiguousarray(x[b, :T].T.astype(NP_BF16)) for b in range(x.shape[0])
    ]
    in_maps = []
    for g in range(N_GROUPS):
        sl = slice(g * DL, (g + 1) * DL)
        wqT = np.ascontiguousarray((Wq[sl] * scale).T.astype(NP_BF16))
        wkT = np.ascontiguousarray(Wk[sl].T.astype(NP_BF16))
        wvT = np.ascontiguousarray(Wv[sl].T.astype(NP_BF16))
        wpT = np.ascontiguousarray(Wp[:, sl].T.astype(NP_BF16))
        for b in range(len(xT)):
            in_maps.append(
                {"xT": xT[b], "wqT": wqT, "wkT": wkT, "wvT": wvT, "wpT": wpT}
            )
    return in_maps


_PROGRAM = None


def kernel(x, Wk, Wq, Wv, Wp, bp):
    global _PROGRAM
    x = np.asarray(x, np.float32)
    bp = np.asarray(bp, np.float32)
    B, T, _ = x.shape

    if _PROGRAM is None:
        _PROGRAM = build_program(T)
    nc = _PROGRAM

    in_maps = shard_inputs(x, Wk, Wq, Wv, Wp, T=T)
    res = run_bass_kernel_spmd(nc, in_maps, core_ids=list(range(N_CORES)))
    parts = [r["out"] for r in res.results]

    out = np.zeros((B, T, C), np.float32)
    for g in range(N_GROUPS):
        for b in range(B):
            out[b] += parts[g * N_BATCH + b]
    out += bp
    return out


# revision 29
# speedup vs baseline: 1.0901x; 1.0901x over previous
"""Trainium2 Bass kernel for CausalSelfAttention (no causal mask in reference).

Problem shapes: x [B=2, T=2048, C=1024], H=16 heads, D=64 head dim.
  q/k/v = x @ W{q,k,v}.T ; att = softmax(q k^T / sqrt(D)) ; y = att v
  out = y @ Wp.T + bp

Sharding over 8 NeuronCores: 4 head-groups (4 heads = 256 dims each) x 2
batches.  Core (g, b) computes a partial output for x[b] restricted to head
group g; the host sums the 4 head-group partials per batch and adds bp.

Per-core device program (matmul operands bf16, fp32 PSUM accumulate):
  1. QT = (Wq_g*scale) @ x^T   [256, T]   (d on partitions, t on free axis)
     KT = Wk_g @ x^T           [256, T]
     V  = x @ Wv_g^T           [T, 256]   (natural layout, + ones columns)
  2. per head h, per 512-wide t-chunk:
       S_T[s, t] = KT_h-tile @ QT_h       (scores transposed: s on partitions;
                                           two heads packed in PE row groups)
       P = exp(S_T)                       (no max subtraction: scores are O(1)
                                           by construction, exp is safe)
       Yaug^T = [V_h | 1...1]^T @ P       -> rows 0..63 unnormalized Y^T,
                                             rows 64..127 = softmax denom
                                             (replicated by the ones columns)
       Y^T = Yaug^T[0:64] * recip(rows 64..127)
  3. out_partial = Y^T-tiles^T @ Wp_g^T   [T, 1024]
All layouts chain with zero on-chip transposes.  Emission order interleaves
phase 1 with attention so the Scalar engine (exp, the throughput floor)
starts early and never starves.
"""

import numpy as np
import ml_dtypes

import concourse.bass as bass
import concourse.tile as tile
from concourse import mybir
from concourse.bacc import Bacc
from concourse.bass_utils import run_bass_kernel_spmd

BF16 = mybir.dt.bfloat16
FP8 = mybir.dt.float8e4
F32 = mybir.dt.float32
NP_BF16 = ml_dtypes.bfloat16

P = 128
C = 1024
H = 16
D = 64
N_CORES = 8
N_GROUPS = 4              # head groups (tensor parallel)
N_BATCH = 2               # data parallel over B
HL = H // N_GROUPS        # 4 local heads
DL = HL * D               # 256 local head dims
CHUNK = 512               # t-chunk width (one PSUM bank of fp32)


def build_program(T: int = 2048) -> bass.Bass:
    KO = C // P            # k-tiles over the C contraction
    TT = T // P            # s/t tiles of 128
    NCH = T // CHUNK       # t-chunks
    KP = DL // P           # k-tiles over local head dims (2)

    nc = Bacc()
    xT_d = nc.declare_dram_parameter("xT", [C, T], BF16, isOutput=False)
    wqT_d = nc.declare_dram_parameter("wqT", [C, DL], BF16, isOutput=False)
    wkT_d = nc.declare_dram_parameter("wkT", [C, DL], BF16, isOutput=False)
    wvT_d = nc.declare_dram_parameter("wvT", [C, DL], BF16, isOutput=False)
    wpT_d = nc.declare_dram_parameter("wpT", [DL, C], BF16, isOutput=False)
    out_d = nc.declare_dram_parameter("out", [T, C], F32, isOutput=True)

    EXP = mybir.ActivationFunctionType.Exp

    with tile.TileContext(nc) as tc:
        with (
            tc.tile_pool(name="const", bufs=1) as cp,
            tc.tile_pool(name="att_s", bufs=2, space="PSUM") as att_s,
            tc.tile_pool(name="accy", bufs=2, space="PSUM") as accy,
            tc.tile_pool(name="accps", bufs=2, space="PSUM") as accps,
            tc.tile_pool(name="expp", bufs=44) as exp_pool,
            tc.tile_pool(name="normp", bufs=3) as norm_pool,
            tc.tile_pool(name="outp", bufs=3) as out_pool,
        ):
            xT_sb = cp.tile([P, KO, T], BF16)
            wqT_sb = cp.tile([P, KO, DL], BF16)
            wkT_sb = cp.tile([P, KO, DL], BF16)
            wvT_sb = cp.tile([P, KO, DL], BF16)
            wpT_sb = cp.tile([P, KP, C], BF16)
            QT_sb = cp.tile([P, KP, T], BF16)
            KT_sb = cp.tile([P, KP, T], BF16)
            # per head: 64 V columns then 64 ones columns; the ones columns
            # make the PV matmul emit the softmax denominator replicated
            # across PSUM partitions 64..127 (partition broadcast for free).
            Vaug_sb = cp.tile([P, TT, HL * 2 * D], BF16)
            YT_sb = cp.tile([P, KP, T], BF16)

            # DMAs ordered by first use: K weights, x, Q/V weights, Wp last
            nc.gpsimd.dma_start(
                out=wkT_sb[:, :, :],
                in_=wkT_d[:, :].rearrange("(ko p) d -> p ko d", p=P),
            )
            # x slabs ordered chunk-major so the first projection group's
            # k-loop unblocks after 1/NCH of the x transfer; slabs spread
            # over four engines' DMA queues so transfers run concurrently
            xT_r = xT_d[:, :].rearrange("(ko p) t -> ko p t", p=P)
            dma_engs = [nc.sync, nc.gpsimd]
            for ch in range(NCH):
                for k in range(KO):
                    dma_engs[ch % 2].dma_start(
                        out=xT_sb[:, k, ch * CHUNK : (ch + 1) * CHUNK],
                        in_=xT_r[k][:, ch * CHUNK : (ch + 1) * CHUNK],
                    )
            for w_d, w_sb in ((wqT_d, wqT_sb), (wvT_d, wvT_sb)):
                nc.sync.dma_start(
                    out=w_sb[:, :, :],
                    in_=w_d[:, :].rearrange("(ko p) d -> p ko d", p=P),
                )
            nc.sync.dma_start(
                out=wpT_sb[:, :, :],
                in_=wpT_d[:, :].rearrange("(kp p) n -> p kp n", p=P),
            )

            vview = Vaug_sb.rearrange("p tt (h e) -> p tt h e", e=2 * D)
            nc.gpsimd.memset(vview[:, :, :, D : 2 * D], 1.0)

            # ---------- emission helpers ----------
            def emit_qk_group(w_sb, o_sb, m, ch):
                ps = accps.tile([P, CHUNK], F32, tag="acc", name="ps")
                for k in range(KO):
                    nc.tensor.matmul(
                        ps,
                        lhsT=w_sb[:, k, m * P : (m + 1) * P],
                        rhs=xT_sb[:, k, ch * CHUNK : (ch + 1) * CHUNK],
                        start=(k == 0),
                        stop=(k == KO - 1),
                    )
                nc.vector.tensor_copy(
                    out=o_sb[:, m, ch * CHUNK : (ch + 1) * CHUNK], in_=ps
                )

            def emit_v_group(m):
                ps = accps.tile([P, CHUNK], F32, tag="acc", name="ps")
                for k in range(KO):
                    nc.tensor.matmul(
                        ps[:, 0:DL],
                        lhsT=xT_sb[:, k, m * P : (m + 1) * P],
                        rhs=wvT_sb[:, k, :],
                        start=(k == 0),
                        stop=(k == KO - 1),
                    )
                nc.vector.tensor_copy(
                    out=vview[:, m, :, 0:D],
                    in_=ps[:, 0:DL].rearrange("p (h e) -> p h e", e=D),
                )

            exps = {}  # (ch, hp) -> list of exp tiles

            def emit_sexp(ch, hp):
                t0 = ch * CHUNK
                lst = []
                for s in range(TT):
                    ps_s = att_s.tile([P, 2 * CHUNK], F32, tag="s", name="ps_s")
                    # two heads packed into PE row groups (K=64 each)
                    nc.tensor.matmul(
                        ps_s[:, 0:CHUNK],
                        lhsT=KT_sb[0:64, hp, s * P : (s + 1) * P],
                        rhs=QT_sb[0:64, hp, t0 : t0 + CHUNK],
                        start=True,
                        stop=True,
                    )
                    nc.tensor.matmul(
                        ps_s[:, CHUNK : 2 * CHUNK],
                        lhsT=KT_sb[64:128, hp, s * P : (s + 1) * P],
                        rhs=QT_sb[64:128, hp, t0 : t0 + CHUNK],
                        start=True,
                        stop=True,
                    )
                    ex = exp_pool.tile([P, 2 * CHUNK], BF16, tag="e", name="ex")
                    nc.scalar.activation(out=ex, in_=ps_s, func=EXP)
                    lst.append(ex)
                exps[(ch, hp)] = lst

            def emit_pv(ch, hp):
                t0 = ch * CHUNK
                lst = exps.pop((ch, hp))
                ps_y = {}
                for ha in range(2):
                    ps_y[ha] = accy.tile([P, CHUNK], F32, tag="y", name="ps_y")
                for s in range(TT):
                    for ha in range(2):
                        h = hp * 2 + ha
                        nc.tensor.matmul(
                            ps_y[ha],
                            lhsT=Vaug_sb[:, s, h * 2 * D : (h + 1) * 2 * D],
                            rhs=lst[s][:, ha * CHUNK : (ha + 1) * CHUNK],
                            start=(s == 0),
                            stop=(s == TT - 1),
                        )
                for ha in range(2):
                    recip = norm_pool.tile([D, CHUNK], F32, tag="r", name="recip")
                    nc.vector.reciprocal(out=recip, in_=ps_y[ha][D : 2 * D, :])
                    nc.vector.tensor_mul(
                        out=YT_sb[ha * D : (ha + 1) * D, hp, t0 : t0 + CHUNK],
                        in0=ps_y[ha][0:D, :],
                        in1=recip,
                    )

            def emit_outproj(ch):
                for mt in range(CHUNK // P):
                    m = ch * (CHUNK // P) + mt
                    for n2 in range(C // CHUNK):
                        ps_o = accps.tile([P, CHUNK], F32, tag="acc", name="ps_o")
                        for kk in range(KP):
                            nc.tensor.matmul(
                                ps_o,
                                lhsT=YT_sb[:, kk, m * P : (m + 1) * P],
                                rhs=wpT_sb[:, kk, n2 * CHUNK : (n2 + 1) * CHUNK],
                                start=(kk == 0),
                                stop=(kk == KP - 1),
                            )
                        o_sb = out_pool.tile([P, CHUNK], F32, tag="o", name="o_sb")
                        nc.vector.tensor_copy(out=o_sb, in_=ps_o)
                        nc.sync.dma_start(
                            out=out_d[
                                m * P : (m + 1) * P,
                                n2 * CHUNK : (n2 + 1) * CHUNK,
                            ],
                            in_=o_sb,
                        )

            # ---------- emission order ----------
            # scores+exp for head-pair hp needs only K tile hp (all chunks)
            # and Q tile hp (that chunk), so the Scalar engine (the
            # throughput floor) starts exp'ing ~15us in; V and the remaining
            # Q chunks fill PE time under those exps, then a lookahead-1
            # software pipeline keeps ACT fed through the PV/proj phases.
            for ch in range(NCH):
                emit_qk_group(wkT_sb, KT_sb, 0, ch)
            emit_qk_group(wqT_sb, QT_sb, 0, 0)
            emit_sexp(0, 0)
            for ch in range(NCH):
                emit_qk_group(wkT_sb, KT_sb, 1, ch)
            emit_qk_group(wqT_sb, QT_sb, 1, 0)
            emit_sexp(0, 1)
            for m in range(TT // 2):
                emit_v_group(m)
            if NCH > 1:
                emit_qk_group(wqT_sb, QT_sb, 0, 1)
                emit_sexp(1, 0)
            for m in range(TT // 2, TT):
                emit_v_group(m)
            if NCH > 1:
                emit_qk_group(wqT_sb, QT_sb, 1, 1)
                emit_sexp(1, 1)
            for ch in range(2, NCH):
                for m in range(KP):
                    emit_qk_group(wqT_sb, QT_sb, m, ch)
            for ch in range(NCH):
                if 2 <= ch + 1 < NCH:
                    emit_sexp(ch + 1, 0)
                emit_pv(ch, 0)
                emit_pv(ch, 1)
                if 2 <= ch + 1 < NCH:
                    emit_sexp(ch + 1, 1)
                emit_outproj(ch)
    # run the Bacc passes (matmul-wait relocation, wait splitting, reg alloc)
    nc.finalize()
    return nc


def shard_inputs(x, Wk, Wq, Wv, Wp, T=2048):
    """Build the 8 per-core input dicts (host-side transposes + bf16 casts)."""
    scale = 1.0 / np.sqrt(np.float32(D))
    x = np.asarray(x, np.float32)
    Wk = np.asarray(Wk, np.float32)
    Wq = np.asarray(Wq, np.float32)
    Wv = np.asarray(Wv, np.float32)
    Wp = np.asarray(Wp, np.float32)

    xT = [
        np.ascontiguousarray(x[b, :T].T.astype(NP_BF16)) for b in range(x.shape[0])
    ]
    in_maps = []
    for g in range(N_GROUPS):
        sl = slice(g * DL, (g + 1) * DL)
        wqT = np.ascontiguousarray((Wq[sl] * scale).T.astype(NP_BF16))
        wkT = np.ascontiguousarray(Wk[sl].T.astype(NP_BF16))
        wvT = np.ascontiguousarray(Wv[sl].T.astype(NP_BF16))
        wpT = np.ascontiguousarray(Wp[:, sl].T.astype(NP_BF16))
        for b in range(len(xT)):
            in_maps.append(
                {"xT": xT[b], "wqT": wqT, "wkT": wkT, "wvT": wvT, "wpT": wpT}
            )
    return in_maps


_PROGRAM = None


def kernel(x, Wk, Wq, Wv, Wp, bp):
    global _PROGRAM
    x = np.asarray(x, np.float32)
    bp = np.asarray(bp, np.float32)
    B, T, _ = x.shape

    if _PROGRAM is None:
        _PROGRAM = build_program(T)
    nc = _PROGRAM

    in_maps = shard_inputs(x, Wk, Wq, Wv, Wp, T=T)
    res = run_bass_kernel_spmd(nc, in_maps, core_ids=list(range(N_CORES)))
    parts = [r["out"] for r in res.results]

    out = np.zeros((B, T, C), np.float32)
    for g in range(N_GROUPS):
        for b in range(B):
            out[b] += parts[g * N_BATCH + b]
    out += bp
    return out


# revision 35
# speedup vs baseline: 1.0954x; 1.0049x over previous
"""Trainium2 Bass kernel for CausalSelfAttention (no causal mask in reference).

Problem shapes: x [B=2, T=2048, C=1024], H=16 heads, D=64 head dim.
  q/k/v = x @ W{q,k,v}.T ; att = softmax(q k^T / sqrt(D)) ; y = att v
  out = y @ Wp.T + bp

Sharding over 8 NeuronCores: 4 head-groups (4 heads = 256 dims each) x 2
batches.  Core (g, b) computes a partial output for x[b] restricted to head
group g; the host sums the 4 head-group partials per batch and adds bp.

Per-core device program (matmul operands bf16, fp32 PSUM accumulate):
  1. QT = (Wq_g*scale) @ x^T   [256, T]   (d on partitions, t on free axis)
     KT = Wk_g @ x^T           [256, T]
     V  = x @ Wv_g^T           [T, 256]   (natural layout, + ones columns)
  2. per head h, per 512-wide t-chunk:
       S_T[s, t] = KT_h-tile @ QT_h       (scores transposed: s on partitions;
                                           two heads packed in PE row groups)
       P = exp(S_T)                       (no max subtraction: scores are O(1)
                                           by construction, exp is safe)
       Yaug^T = [V_h | 1...1]^T @ P       -> rows 0..63 unnormalized Y^T,
                                             rows 64..127 = softmax denom
                                             (replicated by the ones columns)
       Y^T = Yaug^T[0:64] * recip(rows 64..127)
  3. out_partial = Y^T-tiles^T @ Wp_g^T   [T, 1024]
All layouts chain with zero on-chip transposes.  Emission order interleaves
phase 1 with attention so the Scalar engine (exp, the throughput floor)
starts early and never starves.
"""

import numpy as np
import ml_dtypes

import concourse.bass as bass
import concourse.tile as tile
from concourse import mybir
from concourse.bacc import Bacc
from concourse.bass_utils import run_bass_kernel_spmd

BF16 = mybir.dt.bfloat16
F32 = mybir.dt.float32
NP_BF16 = ml_dtypes.bfloat16

P = 128
C = 1024
H = 16
D = 64
N_CORES = 8
N_GROUPS = 4              # head groups (tensor parallel)
N_BATCH = 2               # data parallel over B
HL = H // N_GROUPS        # 4 local heads
DL = HL * D               # 256 local head dims
CHUNK = 512               # t-chunk width (one PSUM bank of fp32)


def build_program(T: int = 2048) -> bass.Bass:
    KO = C // P            # k-tiles over the C contraction
    TT = T // P            # s/t tiles of 128
    NCH = T // CHUNK       # t-chunks
    KP = DL // P           # k-tiles over local head dims (2)

    nc = Bacc()
    xT_d = nc.declare_dram_parameter("xT", [C, T], BF16, isOutput=False)
    wqT_d = nc.declare_dram_parameter("wqT", [C, DL], BF16, isOutput=False)
    wkT_d = nc.declare_dram_parameter("wkT", [C, DL], BF16, isOutput=False)
    wvT_d = nc.declare_dram_parameter("wvT", [C, DL], BF16, isOutput=False)
    wpT_d = nc.declare_dram_parameter("wpT", [DL, C], BF16, isOutput=False)
    out_d = nc.declare_dram_parameter("out", [T, C], F32, isOutput=True)

    EXP = mybir.ActivationFunctionType.Exp

    with tile.TileContext(nc) as tc:
        with (
            tc.tile_pool(name="const", bufs=1) as cp,
            tc.tile_pool(name="att_s", bufs=2, space="PSUM") as att_s,
            tc.tile_pool(name="accy", bufs=2, space="PSUM") as accy,
            tc.tile_pool(name="accps", bufs=2, space="PSUM") as accps,
            tc.tile_pool(name="expp", bufs=44) as exp_pool,
            tc.tile_pool(name="normp", bufs=4) as norm_pool,
            tc.tile_pool(name="outp", bufs=4) as out_pool,
        ):
            xT_sb = cp.tile([P, KO, T], BF16)
            wqT_sb = cp.tile([P, KO, DL], BF16)
            wkT_sb = cp.tile([P, KO, DL], BF16)
            wvT_sb = cp.tile([P, KO, DL], BF16)
            wpT_sb = cp.tile([P, KP, C], BF16)
            QT_sb = cp.tile([P, KP, T], BF16)
            KT_sb = cp.tile([P, KP, T], BF16)
            # per head: 64 V columns then 64 ones columns; the ones columns
            # make the PV matmul emit the softmax denominator replicated
            # across PSUM partitions 64..127 (partition broadcast for free).
            Vaug_sb = cp.tile([P, TT, HL * 2 * D], BF16)
            YT_sb = cp.tile([P, KP, T], BF16)

            # DMAs ordered by first use: K weights, x, Q/V weights, Wp last
            nc.gpsimd.dma_start(
                out=wkT_sb[:, :, :],
                in_=wkT_d[:, :].rearrange("(ko p) d -> p ko d", p=P),
            )
            # x slabs ordered chunk-major so the first projection group's
            # k-loop unblocks after 1/NCH of the x transfer; slabs spread
            # over two engines' DMA queues so transfers run concurrently
            xT_r = xT_d[:, :].rearrange("(ko p) t -> ko p t", p=P)
            dma_engs = [nc.sync, nc.gpsimd]
            for ch in range(NCH):
                for k in range(KO):
                    dma_engs[ch % 2].dma_start(
                        out=xT_sb[:, k, ch * CHUNK : (ch + 1) * CHUNK],
                        in_=xT_r[k][:, ch * CHUNK : (ch + 1) * CHUNK],
                    )
            for w_d, w_sb in ((wqT_d, wqT_sb), (wvT_d, wvT_sb)):
                nc.sync.dma_start(
                    out=w_sb[:, :, :],
                    in_=w_d[:, :].rearrange("(ko p) d -> p ko d", p=P),
                )
            nc.sync.dma_start(
                out=wpT_sb[:, :, :],
                in_=wpT_d[:, :].rearrange("(kp p) n -> p kp n", p=P),
            )

            vview = Vaug_sb.rearrange("p tt (h e) -> p tt h e", e=2 * D)
            nc.gpsimd.memset(vview[:, :, :, D : 2 * D], 1.0)

            # ---------- emission helpers ----------
            def emit_qk_group(w_sb, o_sb, m, ch):
                ps = accps.tile([P, CHUNK], F32, tag="acc", name="ps")
                for k in range(KO):
                    nc.tensor.matmul(
                        ps,
                        lhsT=w_sb[:, k, m * P : (m + 1) * P],
                        rhs=xT_sb[:, k, ch * CHUNK : (ch + 1) * CHUNK],
                        start=(k == 0),
                        stop=(k == KO - 1),
                    )
                nc.vector.tensor_copy(
                    out=o_sb[:, m, ch * CHUNK : (ch + 1) * CHUNK], in_=ps
                )

            def emit_v_group(m):
                ps = accps.tile([P, CHUNK], F32, tag="acc", name="ps")
                for k in range(KO):
                    nc.tensor.matmul(
                        ps[:, 0:DL],
                        lhsT=xT_sb[:, k, m * P : (m + 1) * P],
                        rhs=wvT_sb[:, k, :],
                        start=(k == 0),
                        stop=(k == KO - 1),
                    )
                nc.vector.tensor_copy(
                    out=vview[:, m, :, 0:D],
                    in_=ps[:, 0:DL].rearrange("p (h e) -> p h e", e=D),
                )

            exps = {}  # (ch, hp) -> list of exp tiles

            def emit_sexp(ch, hp):
                t0 = ch * CHUNK
                lst = []
                for s in range(TT):
                    ps_s = att_s.tile([P, 2 * CHUNK], F32, tag="s", name="ps_s")
                    # two heads packed into PE row groups (K=64 each)
                    nc.tensor.matmul(
                        ps_s[:, 0:CHUNK],
                        lhsT=KT_sb[0:64, hp, s * P : (s + 1) * P],
                        rhs=QT_sb[0:64, hp, t0 : t0 + CHUNK],
                        start=True,
                        stop=True,
                    )
                    nc.tensor.matmul(
                        ps_s[:, CHUNK : 2 * CHUNK],
                        lhsT=KT_sb[64:128, hp, s * P : (s + 1) * P],
                        rhs=QT_sb[64:128, hp, t0 : t0 + CHUNK],
                        start=True,
                        stop=True,
                    )
                    ex = exp_pool.tile([P, 2 * CHUNK], BF16, tag="e", name="ex")
                    nc.scalar.activation(out=ex, in_=ps_s, func=EXP)
                    lst.append(ex)
                exps[(ch, hp)] = lst

            def emit_pv(ch, hp):
                t0 = ch * CHUNK
                lst = exps.pop((ch, hp))
                ps_y = {}
                for ha in range(2):
                    ps_y[ha] = accy.tile([P, CHUNK], F32, tag="y", name="ps_y")
                for s in range(TT):
                    for ha in range(2):
                        h = hp * 2 + ha
                        nc.tensor.matmul(
                            ps_y[ha],
                            lhsT=Vaug_sb[:, s, h * 2 * D : (h + 1) * 2 * D],
                            rhs=lst[s][:, ha * CHUNK : (ha + 1) * CHUNK],
                            start=(s == 0),
                            stop=(s == TT - 1),
                        )
                for ha in range(2):
                    recip = norm_pool.tile([D, CHUNK], F32, tag="r", name="recip")
                    nc.vector.reciprocal(out=recip, in_=ps_y[ha][D : 2 * D, :])
                    nc.vector.tensor_mul(
                        out=YT_sb[ha * D : (ha + 1) * D, hp, t0 : t0 + CHUNK],
                        in0=ps_y[ha][0:D, :],
                        in1=recip,
                    )

            def emit_outproj(ch):
                for mt in range(CHUNK // P):
                    m = ch * (CHUNK // P) + mt
                    for n2 in range(C // CHUNK):
                        ps_o = accps.tile([P, CHUNK], F32, tag="acc", name="ps_o")
                        for kk in range(KP):
                            nc.tensor.matmul(
                                ps_o,
                                lhsT=YT_sb[:, kk, m * P : (m + 1) * P],
                                rhs=wpT_sb[:, kk, n2 * CHUNK : (n2 + 1) * CHUNK],
                                start=(kk == 0),
                                stop=(kk == KP - 1),
                            )
                        o_sb = out_pool.tile([P, CHUNK], F32, tag="o", name="o_sb")
                        nc.vector.tensor_copy(out=o_sb, in_=ps_o)
                        nc.sync.dma_start(
                            out=out_d[
                                m * P : (m + 1) * P,
                                n2 * CHUNK : (n2 + 1) * CHUNK,
                            ],
                            in_=o_sb,
                        )

            # ---------- emission order ----------
            # scores+exp for head-pair hp needs only K tile hp (all chunks)
            # and Q tile hp (that chunk), so the Scalar engine (the
            # throughput floor) starts exp'ing ~15us in; V and the remaining
            # Q chunks fill PE time under those exps, then a lookahead-1
            # software pipeline keeps ACT fed through the PV/proj phases.
            for ch in range(NCH):
                emit_qk_group(wkT_sb, KT_sb, 0, ch)
            emit_qk_group(wqT_sb, QT_sb, 0, 0)
            emit_sexp(0, 0)
            for ch in range(NCH):
                emit_qk_group(wkT_sb, KT_sb, 1, ch)
            emit_qk_group(wqT_sb, QT_sb, 1, 0)
            emit_sexp(0, 1)
            for m in range(TT // 2):
                emit_v_group(m)
            if NCH > 1:
                emit_qk_group(wqT_sb, QT_sb, 0, 1)
                emit_sexp(1, 0)
            for m in range(TT // 2, TT):
                emit_v_group(m)
            if NCH > 1:
                emit_qk_group(wqT_sb, QT_sb, 1, 1)
                emit_sexp(1, 1)
            for ch in range(2, NCH):
                for m in range(KP):
                    emit_qk_group(wqT_sb, QT_sb, m, ch)
            for ch in range(NCH):
                if 2 <= ch + 1 < NCH:
                    emit_sexp(ch + 1, 0)
                emit_pv(ch, 0)
                emit_pv(ch, 1)
                if 2 <= ch + 1 < NCH:
                    emit_sexp(ch + 1, 1)
                emit_outproj(ch)
    # run the Bacc passes (matmul-wait relocation, wait splitting, reg alloc)
    nc.finalize()
    return nc


def shard_inputs(x, Wk, Wq, Wv, Wp, T=2048):
    """Build the 8 per-core input dicts (host-side transposes + bf16 casts)."""
    scale = 1.0 / np.sqrt(np.float32(D))
    x = np.asarray(x, np.float32)
    Wk = np.asarray(Wk, np.float32)
    Wq = np.asarray(Wq, np.float32)
    Wv = np.asarray(Wv, np.float32)
    Wp = np.asarray(Wp, np.float32)

    xT = [
        np.ascontiguousarray(x[b, :T].T.astype(NP_BF16)) for b in range(x.shape[0])
    ]
    in_maps = []
    for g in range(N_GROUPS):
        sl = slice(g * DL, (g + 1) * DL)
        wqT = np.ascontiguousarray((Wq[sl] * scale).T.astype(NP_BF16))
        wkT = np.ascontiguousarray(Wk[sl].T.astype(NP_BF16))
        wvT = np.ascontiguousarray(Wv[sl].T.astype(NP_BF16))
        wpT = np.ascontiguousarray(Wp[:, sl].T.astype(NP_BF16))
        for b in range(len(xT)):
            in_maps.append(
                {"xT": xT[b], "wqT": wqT, "wkT": wkT, "wvT": wvT, "wpT": wpT}
            )
    return in_maps


_PROGRAM = None


def kernel(x, Wk, Wq, Wv, Wp, bp):
    global _PROGRAM
    x = np.asarray(x, np.float32)
    bp = np.asarray(bp, np.float32)
    B, T, _ = x.shape

    if _PROGRAM is None:
        _PROGRAM = build_program(T)
    nc = _PROGRAM

    in_maps = shard_inputs(x, Wk, Wq, Wv, Wp, T=T)
    res = run_bass_kernel_spmd(nc, in_maps, core_ids=list(range(N_CORES)))
    parts = [r["out"] for r in res.results]

    out = np.zeros((B, T, C), np.float32)
    for g in range(N_GROUPS):
        for b in range(B):
            out[b] += parts[g * N_BATCH + b]
    out += bp
    return out


# revision 43
# speedup vs baseline: 1.1511x; 1.0509x over previous
"""Trainium2 Bass kernel for CausalSelfAttention (no causal mask in reference).

Problem shapes: x [B=2, T=2048, C=1024], H=16 heads, D=64 head dim.
  q/k/v = x @ W{q,k,v}.T ; att = softmax(q k^T / sqrt(D)) ; y = att v
  out = y @ Wp.T + bp

Sharding over 8 NeuronCores: 4 head-groups (4 heads = 256 dims each) x 2
batches.  Core (g, b) computes a partial output for x[b] restricted to head
group g; the host sums the 4 head-group partials per batch and adds bp.

Per-core device program (matmul operands bf16, fp32 PSUM accumulate):
  1. QT = (Wq_g*scale) @ x^T   [256, T]   (d on partitions, t on free axis)
     KT = Wk_g @ x^T           [256, T]
     V  = x @ Wv_g^T           [T, 256]   (natural layout, + ones columns)
  2. per head h, per 512-wide t-chunk:
       S_T[s, t] = KT_h-tile @ QT_h       (scores transposed: s on partitions;
                                           two heads packed in PE row groups)
       P = exp(S_T)                       (no max subtraction: scores are O(1)
                                           by construction, exp is safe)
       Yaug^T = [V_h | 1...1]^T @ P       -> rows 0..63 unnormalized Y^T,
                                             rows 64..127 = softmax denom
                                             (replicated by the ones columns)
       Y^T = Yaug^T[0:64] * recip(rows 64..127)
  3. out_partial = Y^T-tiles^T @ Wp_g^T   [T, 1024]
All layouts chain with zero on-chip transposes.  Emission order interleaves
phase 1 with attention so the Scalar engine (exp, the throughput floor)
starts early and never starves.
"""

import numpy as np
import ml_dtypes

import concourse.bass as bass
import concourse.tile as tile
from concourse import mybir
from concourse.bacc import Bacc
from concourse.bass_utils import run_bass_kernel_spmd

BF16 = mybir.dt.bfloat16
F32 = mybir.dt.float32
NP_BF16 = ml_dtypes.bfloat16

P = 128
C = 1024
H = 16
D = 64
N_CORES = 8
N_GROUPS = 4              # head groups (tensor parallel)
N_BATCH = 2               # data parallel over B
HL = H // N_GROUPS        # 4 local heads
DL = HL * D               # 256 local head dims
CHUNK = 512               # t-chunk width (one PSUM bank of fp32)


def build_program(T: int = 2048) -> bass.Bass:
    KO = C // P            # k-tiles over the C contraction
    TT = T // P            # s/t tiles of 128
    NCH = T // CHUNK       # t-chunks
    KP = DL // P           # k-tiles over local head dims (2)

    nc = Bacc()
    xT_d = nc.declare_dram_parameter("xT", [C, T], BF16, isOutput=False)
    wqT_d = nc.declare_dram_parameter("wqT", [C, DL], BF16, isOutput=False)
    wkT_d = nc.declare_dram_parameter("wkT", [C, DL], BF16, isOutput=False)
    wvT_d = nc.declare_dram_parameter("wvT", [C, DL], BF16, isOutput=False)
    wpT_d = nc.declare_dram_parameter("wpT", [DL, C], BF16, isOutput=False)
    out_d = nc.declare_dram_parameter("out", [T, C], F32, isOutput=True)

    EXP = mybir.ActivationFunctionType.Exp

    with tile.TileContext(nc) as tc:
        with (
            tc.tile_pool(name="const", bufs=1) as cp,
            tc.tile_pool(name="att_s", bufs=2, space="PSUM") as att_s,
            tc.tile_pool(name="accy", bufs=2, space="PSUM") as accy,
            tc.tile_pool(name="accps", bufs=2, space="PSUM") as accps,
            tc.tile_pool(name="expp", bufs=44) as exp_pool,
            tc.tile_pool(name="normp", bufs=4) as norm_pool,
            tc.tile_pool(name="outp", bufs=4) as out_pool,
        ):
            xT_sb = cp.tile([P, KO, T], BF16)
            wqT_sb = cp.tile([P, KO, DL], BF16)
            wkT_sb = cp.tile([P, KO, DL], BF16)
            wvT_sb = cp.tile([P, KO, DL], BF16)
            wpT_sb = cp.tile([P, KP, C], BF16)
            QT_sb = cp.tile([P, KP, T], BF16)
            KT_sb = cp.tile([P, KP, T], BF16)
            # per head: 64 V columns then 64 ones columns; the ones columns
            # make the PV matmul emit the softmax denominator replicated
            # across PSUM partitions 64..127 (partition broadcast for free).
            Vaug_sb = cp.tile([P, TT, HL * 2 * D], BF16)
            YT_sb = cp.tile([P, KP, T], BF16)

            # DMAs ordered by first use: K weights, x, Q/V weights, Wp last
            wkT_r = wkT_d[:, :].rearrange("(ko p) d -> p ko d", p=P)
            nc.gpsimd.dma_start(out=wkT_sb[:, 0:4, :], in_=wkT_r[:, 0:4, :])
            nc.gpsimd.dma_start(out=wkT_sb[:, 4:8, :], in_=wkT_r[:, 4:8, :])
            # x slabs ordered chunk-major so the first projection group's
            # k-loop unblocks after 1/NCH of the x transfer; slabs spread
            # over two engines' DMA queues so transfers run concurrently
            xT_r = xT_d[:, :].rearrange("(ko p) t -> ko p t", p=P)
            dma_engs = [nc.sync, nc.gpsimd]
            for ch in range(NCH):
                for k in range(KO):
                    dma_engs[ch % 2].dma_start(
                        out=xT_sb[:, k, ch * CHUNK : (ch + 1) * CHUNK],
                        in_=xT_r[k][:, ch * CHUNK : (ch + 1) * CHUNK],
                    )
            for w_d, w_sb in ((wqT_d, wqT_sb), (wvT_d, wvT_sb)):
                nc.sync.dma_start(
                    out=w_sb[:, :, :],
                    in_=w_d[:, :].rearrange("(ko p) d -> p ko d", p=P),
                )
            nc.sync.dma_start(
                out=wpT_sb[:, :, :],
                in_=wpT_d[:, :].rearrange("(kp p) n -> p kp n", p=P),
            )

            vview = Vaug_sb.rearrange("p tt (h e) -> p tt h e", e=2 * D)
            nc.gpsimd.memset(vview[:, :, :, D : 2 * D], 1.0)

            # ---------- emission helpers ----------
            def emit_qk_group(w_sb, o_sb, m, ch):
                ps = accps.tile([P, CHUNK], F32, tag="acc", name="ps")
                for k in range(KO):
                    nc.tensor.matmul(
                        ps,
                        lhsT=w_sb[:, k, m * P : (m + 1) * P],
                        rhs=xT_sb[:, k, ch * CHUNK : (ch + 1) * CHUNK],
                        start=(k == 0),
                        stop=(k == KO - 1),
                    )
                nc.vector.tensor_copy(
                    out=o_sb[:, m, ch * CHUNK : (ch + 1) * CHUNK], in_=ps
                )

            def emit_v_group(m):
                ps = accps.tile([P, CHUNK], F32, tag="acc", name="ps")
                for k in range(KO):
                    nc.tensor.matmul(
                        ps[:, 0:DL],
                        lhsT=xT_sb[:, k, m * P : (m + 1) * P],
                        rhs=wvT_sb[:, k, :],
                        start=(k == 0),
                        stop=(k == KO - 1),
                    )
                nc.vector.tensor_copy(
                    out=vview[:, m, :, 0:D],
                    in_=ps[:, 0:DL].rearrange("p (h e) -> p h e", e=D),
                )

            exps = {}  # (ch, hp) -> list of exp tiles

            def emit_sexp(ch, hp):
                t0 = ch * CHUNK
                lst = []
                for s in range(TT):
                    ps_s = att_s.tile([P, 2 * CHUNK], F32, tag="s", name="ps_s")
                    # two heads packed into PE row groups (K=64 each)
                    nc.tensor.matmul(
                        ps_s[:, 0:CHUNK],
                        lhsT=KT_sb[0:64, hp, s * P : (s + 1) * P],
                        rhs=QT_sb[0:64, hp, t0 : t0 + CHUNK],
                        start=True,
                        stop=True,
                    )
                    nc.tensor.matmul(
                        ps_s[:, CHUNK : 2 * CHUNK],
                        lhsT=KT_sb[64:128, hp, s * P : (s + 1) * P],
                        rhs=QT_sb[64:128, hp, t0 : t0 + CHUNK],
                        start=True,
                        stop=True,
                    )
                    ex = exp_pool.tile([P, 2 * CHUNK], BF16, tag="e", name="ex")
                    nc.scalar.activation(out=ex, in_=ps_s, func=EXP)
                    lst.append(ex)
                exps[(ch, hp)] = lst

            def emit_pv(ch, hp):
                t0 = ch * CHUNK
                lst = exps.pop((ch, hp))
                ps_y = {}
                for ha in range(2):
                    ps_y[ha] = accy.tile([P, CHUNK], F32, tag="y", name="ps_y")
                for ha in range(2):
                    h = hp * 2 + ha
                    for s in range(TT):
                        nc.tensor.matmul(
                            ps_y[ha],
                            lhsT=Vaug_sb[:, s, h * 2 * D : (h + 1) * 2 * D],
                            rhs=lst[s][:, ha * CHUNK : (ha + 1) * CHUNK],
                            start=(s == 0),
                            stop=(s == TT - 1),
                        )
                    recip = norm_pool.tile([D, CHUNK], F32, tag="r", name="recip")
                    nc.vector.reciprocal(out=recip, in_=ps_y[ha][D : 2 * D, :])
                    nc.vector.tensor_mul(
                        out=YT_sb[ha * D : (ha + 1) * D, hp, t0 : t0 + CHUNK],
                        in0=ps_y[ha][0:D, :],
                        in1=recip,
                    )

            def emit_outproj(ch, last=False):
                for mt in range(CHUNK // P):
                    m = ch * (CHUNK // P) + mt
                    for n2 in range(C // CHUNK):
                        ps_o = accps.tile([P, CHUNK], F32, tag="acc", name="ps_o")
                        for kk in range(KP):
                            nc.tensor.matmul(
                                ps_o,
                                lhsT=YT_sb[:, kk, m * P : (m + 1) * P],
                                rhs=wpT_sb[:, kk, n2 * CHUNK : (n2 + 1) * CHUNK],
                                start=(kk == 0),
                                stop=(kk == KP - 1),
                            )
                        o_sb = out_pool.tile([P, CHUNK], F32, tag="o", name="o_sb")
                        # in the tail the exp stream is done, so the Scalar
                        # engine is free to take half the drain copies
                        if last and n2 % 2 == 0:
                            nc.scalar.copy(out=o_sb, in_=ps_o)
                        else:
                            nc.vector.tensor_copy(out=o_sb, in_=ps_o)
                        nc.sync.dma_start(
                            out=out_d[
                                m * P : (m + 1) * P,
                                n2 * CHUNK : (n2 + 1) * CHUNK,
                            ],
                            in_=o_sb,
                        )

            # ---------- emission order ----------
            # scores+exp for head-pair hp needs only K tile hp (all chunks)
            # and Q tile hp (that chunk), so the Scalar engine (the
            # throughput floor) starts exp'ing ~15us in; V and the remaining
            # Q chunks fill PE time under those exps, then a lookahead-1
            # software pipeline keeps ACT fed through the PV/proj phases.
            for ch in range(NCH):
                emit_qk_group(wkT_sb, KT_sb, 0, ch)
            emit_qk_group(wqT_sb, QT_sb, 0, 0)
            emit_sexp(0, 0)
            for ch in range(NCH):
                emit_qk_group(wkT_sb, KT_sb, 1, ch)
            emit_qk_group(wqT_sb, QT_sb, 1, 0)
            emit_sexp(0, 1)
            for m in range(TT // 2):
                emit_v_group(m)
            if NCH > 1:
                emit_qk_group(wqT_sb, QT_sb, 0, 1)
                emit_sexp(1, 0)
            for m in range(TT // 2, TT):
                emit_v_group(m)
            if NCH > 1:
                emit_qk_group(wqT_sb, QT_sb, 1, 1)
                emit_sexp(1, 1)
            for ch in range(2, NCH):
                for m in range(KP):
                    emit_qk_group(wqT_sb, QT_sb, m, ch)
            # output projection deferred by one chunk: it becomes PE filler
            # work for the stretches where PV is paced by the exp drain
            for ch in range(NCH):
                if 2 <= ch + 1 < NCH:
                    emit_sexp(ch + 1, 0)
                emit_pv(ch, 0)
                if ch >= 1:
                    emit_outproj(ch - 1)
                emit_pv(ch, 1)
                if 2 <= ch + 1 < NCH:
                    emit_sexp(ch + 1, 1)
            emit_outproj(NCH - 1, last=True)
    # run the Bacc passes (matmul-wait relocation, wait splitting, reg alloc)
    nc.finalize()
    return nc


def shard_inputs(x, Wk, Wq, Wv, Wp, T=2048):
    """Build the 8 per-core input dicts (host-side transposes + bf16 casts)."""
    scale = 1.0 / np.sqrt(np.float32(D))
    x = np.asarray(x, np.float32)
    Wk = np.asarray(Wk, np.float32)
    Wq = np.asarray(Wq, np.float32)
    Wv = np.asarray(Wv, np.float32)
    Wp = np.asarray(Wp, np.float32)

    xT = [
        np.ascontiguousarray(x[b, :T].T.astype(NP_BF16)) for b in range(x.shape[0])
    ]
    in_maps = []
    for g in range(N_GROUPS):
        sl = slice(g * DL, (g + 1) * DL)
        wqT = np.ascontiguousarray((Wq[sl] * scale).T.astype(NP_BF16))
        wkT = np.ascontiguousarray(Wk[sl].T.astype(NP_BF16))
        wvT = np.ascontiguousarray(Wv[sl].T.astype(NP_BF16))
        wpT = np.ascontiguousarray(Wp[:, sl].T.astype(NP_BF16))
        for b in range(len(xT)):
            in_maps.append(
                {"xT": xT[b], "wqT": wqT, "wkT": wkT, "wvT": wvT, "wpT": wpT}
            )
    return in_maps


_PROGRAM = None


def kernel(x, Wk, Wq, Wv, Wp, bp):
    global _PROGRAM
    x = np.asarray(x, np.float32)
    bp = np.asarray(bp, np.float32)
    B, T, _ = x.shape

    if _PROGRAM is None:
        _PROGRAM = build_program(T)
    nc = _PROGRAM

    in_maps = shard_inputs(x, Wk, Wq, Wv, Wp, T=T)
    res = run_bass_kernel_spmd(nc, in_maps, core_ids=list(range(N_CORES)))
    parts = [r["out"] for r in res.results]

    out = np.zeros((B, T, C), np.float32)
    for g in range(N_GROUPS):
        for b in range(B):
            out[b] += parts[g * N_BATCH + b]
    out += bp
    return out


# revision 50
# speedup vs baseline: 1.1589x; 1.0068x over previous
"""Trainium2 Bass kernel for CausalSelfAttention (no causal mask in reference).

Problem shapes: x [B=2, T=2048, C=1024], H=16 heads, D=64 head dim.
  q/k/v = x @ W{q,k,v}.T ; att = softmax(q k^T / sqrt(D)) ; y = att v
  out = y @ Wp.T + bp

Sharding over 8 NeuronCores: 4 head-groups (4 heads = 256 dims each) x 2
batches.  Core (g, b) computes a partial output for x[b] restricted to head
group g; the host sums the 4 head-group partials per batch and adds bp.

Per-core device program (matmul operands bf16, fp32 PSUM accumulate):
  1. QT = (Wq_g*scale) @ x^T   [256, T]   (d on partitions, t on free axis)
     KT = Wk_g @ x^T           [256, T]
     V  = x @ Wv_g^T           [T, 256]   (natural layout, + ones columns)
  2. per head h, per 512-wide t-chunk:
       S_T[s, t] = KT_h-tile @ QT_h       (scores transposed: s on partitions;
                                           two heads packed in PE row groups)
       P = exp(S_T)                       (no max subtraction: scores are O(1)
                                           by construction, exp is safe)
       Yaug^T = [V_h | 1...1]^T @ P       -> rows 0..63 unnormalized Y^T,
                                             rows 64..127 = softmax denom
                                             (replicated by the ones columns)
       Y^T = Yaug^T[0:64] * recip(rows 64..127)
  3. out_partial = Y^T-tiles^T @ Wp_g^T   [T, 1024]
All layouts chain with zero on-chip transposes.  Emission order interleaves
phase 1 with attention so the Scalar engine (exp, the throughput floor)
starts early and never starves.
"""

import numpy as np
import ml_dtypes

import concourse.bass as bass
import concourse.tile as tile
from concourse import mybir
from concourse.bacc import Bacc
from concourse.bass_utils import run_bass_kernel_spmd

BF16 = mybir.dt.bfloat16
F32 = mybir.dt.float32
NP_BF16 = ml_dtypes.bfloat16

P = 128
C = 1024
H = 16
D = 64
N_CORES = 8
N_GROUPS = 4              # head groups (tensor parallel)
N_BATCH = 2               # data parallel over B
HL = H // N_GROUPS        # 4 local heads
DL = HL * D               # 256 local head dims
CHUNK = 512               # t-chunk width (one PSUM bank of fp32)


def build_program(T: int = 2048) -> bass.Bass:
    KO = C // P            # k-tiles over the C contraction
    TT = T // P            # s/t tiles of 128
    NCH = T // CHUNK       # t-chunks
    KP = DL // P           # k-tiles over local head dims (2)

    nc = Bacc()
    xT_d = nc.declare_dram_parameter("xT", [C, T], BF16, isOutput=False)
    wqT_d = nc.declare_dram_parameter("wqT", [C, DL], BF16, isOutput=False)
    wkT_d = nc.declare_dram_parameter("wkT", [C, DL], BF16, isOutput=False)
    wvT_d = nc.declare_dram_parameter("wvT", [C, DL], BF16, isOutput=False)
    wpT_d = nc.declare_dram_parameter("wpT", [DL, C], BF16, isOutput=False)
    out_d = nc.declare_dram_parameter("out", [T, C], F32, isOutput=True)

    EXP = mybir.ActivationFunctionType.Exp

    with tile.TileContext(nc) as tc:
        with (
            tc.tile_pool(name="const", bufs=1) as cp,
            tc.tile_pool(name="att_s", bufs=2, space="PSUM") as att_s,
            tc.tile_pool(name="accy", bufs=2, space="PSUM") as accy,
            tc.tile_pool(name="accps", bufs=2, space="PSUM") as accps,
            tc.tile_pool(name="expp", bufs=40) as exp_pool,
            tc.tile_pool(name="normp", bufs=4) as norm_pool,
            tc.tile_pool(name="outp", bufs=4) as out_pool,
        ):
            xT_sb = cp.tile([P, KO, T], BF16)
            wqT_sb = cp.tile([P, KO, DL], BF16)
            wkT_sb = cp.tile([P, KO, DL], BF16)
            wvT_sb = cp.tile([P, KO, DL], BF16)
            wpT_sb = cp.tile([P, KP, C], BF16)
            QT_sb = cp.tile([P, KP, T], BF16)
            KT_sb = cp.tile([P, KP, T], BF16)
            # per head: 64 V columns then 64 ones columns; the ones columns
            # make the PV matmul emit the softmax denominator replicated
            # across PSUM partitions 64..127 (partition broadcast for free).
            Vaug_sb = cp.tile([P, TT, HL * 2 * D], BF16)
            YT_sb = cp.tile([P, KP, T], BF16)

            # DMAs ordered by first use: K weights, x, Q/V weights, Wp last
            wkT_r = wkT_d[:, :].rearrange("(ko p) d -> p ko d", p=P)
            nc.gpsimd.dma_start(out=wkT_sb[:, 0:4, :], in_=wkT_r[:, 0:4, :])
            nc.gpsimd.dma_start(out=wkT_sb[:, 4:8, :], in_=wkT_r[:, 4:8, :])
            # x slabs ordered chunk-major so the first projection group's
            # k-loop unblocks after 1/NCH of the x transfer; slabs spread
            # over two engines' DMA queues so transfers run concurrently
            xT_r = xT_d[:, :].rearrange("(ko p) t -> ko p t", p=P)
            dma_engs = [nc.sync, nc.gpsimd]
            for ch in range(NCH):
                for k in range(KO):
                    dma_engs[ch % 2].dma_start(
                        out=xT_sb[:, k, ch * CHUNK : (ch + 1) * CHUNK],
                        in_=xT_r[k][:, ch * CHUNK : (ch + 1) * CHUNK],
                    )
            for w_d, w_sb in ((wqT_d, wqT_sb), (wvT_d, wvT_sb)):
                nc.sync.dma_start(
                    out=w_sb[:, :, :],
                    in_=w_d[:, :].rearrange("(ko p) d -> p ko d", p=P),
                )
            nc.sync.dma_start(
                out=wpT_sb[:, :, :],
                in_=wpT_d[:, :].rearrange("(kp p) n -> p kp n", p=P),
            )

            vview = Vaug_sb.rearrange("p tt (h e) -> p tt h e", e=2 * D)
            nc.gpsimd.memset(vview[:, :, :, D : 2 * D], 1.0)

            # ---------- emission helpers ----------
            def emit_qk_group(w_sb, o_sb, m, ch):
                ps = accps.tile([P, CHUNK], F32, tag="acc", name="ps")
                for k in range(KO):
                    nc.tensor.matmul(
                        ps,
                        lhsT=w_sb[:, k, m * P : (m + 1) * P],
                        rhs=xT_sb[:, k, ch * CHUNK : (ch + 1) * CHUNK],
                        start=(k == 0),
                        stop=(k == KO - 1),
                    )
                nc.vector.tensor_copy(
                    out=o_sb[:, m, ch * CHUNK : (ch + 1) * CHUNK], in_=ps
                )

            def emit_v_group(m):
                ps = accps.tile([P, CHUNK], F32, tag="acc", name="ps")
                for k in range(KO):
                    nc.tensor.matmul(
                        ps[:, 0:DL],
                        lhsT=xT_sb[:, k, m * P : (m + 1) * P],
                        rhs=wvT_sb[:, k, :],
                        start=(k == 0),
                        stop=(k == KO - 1),
                    )
                nc.vector.tensor_copy(
                    out=vview[:, m, :, 0:D],
                    in_=ps[:, 0:DL].rearrange("p (h e) -> p h e", e=D),
                )

            exps = {}  # (ch, hp) -> list of exp tiles

            def emit_sexp(ch, hp):
                t0 = ch * CHUNK
                lst = []
                for s in range(TT):
                    ps_s = att_s.tile([P, 2 * CHUNK], F32, tag="s", name="ps_s")
                    # two heads packed into PE row groups (K=64 each)
                    nc.tensor.matmul(
                        ps_s[:, 0:CHUNK],
                        lhsT=KT_sb[0:64, hp, s * P : (s + 1) * P],
                        rhs=QT_sb[0:64, hp, t0 : t0 + CHUNK],
                        start=True,
                        stop=True,
                    )
                    nc.tensor.matmul(
                        ps_s[:, CHUNK : 2 * CHUNK],
                        lhsT=KT_sb[64:128, hp, s * P : (s + 1) * P],
                        rhs=QT_sb[64:128, hp, t0 : t0 + CHUNK],
                        start=True,
                        stop=True,
                    )
                    ex = exp_pool.tile([P, 2 * CHUNK], BF16, tag="e", name="ex")
                    nc.scalar.activation(out=ex, in_=ps_s, func=EXP)
                    lst.append(ex)
                exps[(ch, hp)] = lst

            def emit_pv(ch, hp):
                t0 = ch * CHUNK
                lst = exps.pop((ch, hp))
                ps_y = {}
                for ha in range(2):
                    ps_y[ha] = accy.tile([P, CHUNK], F32, tag="y", name="ps_y")
                for ha in range(2):
                    h = hp * 2 + ha
                    for s in range(TT):
                        nc.tensor.matmul(
                            ps_y[ha],
                            lhsT=Vaug_sb[:, s, h * 2 * D : (h + 1) * 2 * D],
                            rhs=lst[s][:, ha * CHUNK : (ha + 1) * CHUNK],
                            start=(s == 0),
                            stop=(s == TT - 1),
                        )
                    recip = norm_pool.tile([D, CHUNK], F32, tag="r", name="recip")
                    nc.vector.reciprocal(out=recip, in_=ps_y[ha][D : 2 * D, :])
                    nc.vector.tensor_mul(
                        out=YT_sb[ha * D : (ha + 1) * D, hp, t0 : t0 + CHUNK],
                        in0=ps_y[ha][0:D, :],
                        in1=recip,
                    )

            def emit_outproj(ch, last=False):
                for mt in range(CHUNK // P):
                    m = ch * (CHUNK // P) + mt
                    for n2 in range(C // CHUNK):
                        ps_o = accps.tile([P, CHUNK], F32, tag="acc", name="ps_o")
                        for kk in range(KP):
                            nc.tensor.matmul(
                                ps_o,
                                lhsT=YT_sb[:, kk, m * P : (m + 1) * P],
                                rhs=wpT_sb[:, kk, n2 * CHUNK : (n2 + 1) * CHUNK],
                                start=(kk == 0),
                                stop=(kk == KP - 1),
                            )
                        o_sb = out_pool.tile([P, CHUNK], F32, tag="o", name="o_sb")
                        # in the tail the exp stream is done, so the Scalar
                        # engine is free to take half the drain copies
                        if last and n2 % 2 == 0:
                            nc.scalar.copy(out=o_sb, in_=ps_o)
                        else:
                            nc.vector.tensor_copy(out=o_sb, in_=ps_o)
                        dma_engs[n2 % 2].dma_start(
                            out=out_d[
                                m * P : (m + 1) * P,
                                n2 * CHUNK : (n2 + 1) * CHUNK,
                            ],
                            in_=o_sb,
                        )

            # ---------- emission order ----------
            # scores+exp for head-pair hp needs only K tile hp (all chunks)
            # and Q tile hp (that chunk), so the Scalar engine (the
            # throughput floor) starts exp'ing ~15us in; V and the remaining
            # Q chunks fill PE time under those exps, then a lookahead-1
            # software pipeline keeps ACT fed through the PV/proj phases.
            for ch in range(NCH):
                emit_qk_group(wkT_sb, KT_sb, 0, ch)
            emit_qk_group(wqT_sb, QT_sb, 0, 0)
            emit_sexp(0, 0)
            for ch in range(NCH):
                emit_qk_group(wkT_sb, KT_sb, 1, ch)
            emit_qk_group(wqT_sb, QT_sb, 1, 0)
            emit_sexp(0, 1)
            for m in range(TT // 2):
                emit_v_group(m)
            if NCH > 1:
                emit_qk_group(wqT_sb, QT_sb, 0, 1)
                emit_sexp(1, 0)
            for m in range(TT // 2, TT):
                emit_v_group(m)
            if NCH > 1:
                emit_qk_group(wqT_sb, QT_sb, 1, 1)
                emit_sexp(1, 1)
            for ch in range(2, NCH):
                for m in range(KP):
                    emit_qk_group(wqT_sb, QT_sb, m, ch)
            # output projection deferred by one chunk: it becomes PE filler
            # work for the stretches where PV is paced by the exp drain
            for ch in range(NCH):
                if 2 <= ch + 1 < NCH:
                    emit_sexp(ch + 1, 0)
                emit_pv(ch, 0)
                if ch >= 1:
                    emit_outproj(ch - 1)
                emit_pv(ch, 1)
                if 2 <= ch + 1 < NCH:
                    emit_sexp(ch + 1, 1)
            emit_outproj(NCH - 1, last=True)
    # run the Bacc passes (matmul-wait relocation, wait splitting, reg alloc)
    nc.finalize()
    return nc


def shard_inputs(x, Wk, Wq, Wv, Wp, T=2048):
    """Build the 8 per-core input dicts (host-side transposes + bf16 casts)."""
    scale = 1.0 / np.sqrt(np.float32(D))
    x = np.asarray(x, np.float32)
    Wk = np.asarray(Wk, np.float32)
    Wq = np.asarray(Wq, np.float32)
    Wv = np.asarray(Wv, np.float32)
    Wp = np.asarray(Wp, np.float32)

    xT = [
        np.ascontiguousarray(x[b, :T].T.astype(NP_BF16)) for b in range(x.shape[0])
    ]
    in_maps = []
    for g in range(N_GROUPS):
        sl = slice(g * DL, (g + 1) * DL)
        wqT = np.ascontiguousarray((Wq[sl] * scale).T.astype(NP_BF16))
        wkT = np.ascontiguousarray(Wk[sl].T.astype(NP_BF16))
        wvT = np.ascontiguousarray(Wv[sl].T.astype(NP_BF16))
        wpT = np.ascontiguousarray(Wp[:, sl].T.astype(NP_BF16))
        for b in range(len(xT)):
            in_maps.append(
                {"xT": xT[b], "wqT": wqT, "wkT": wkT, "wvT": wvT, "wpT": wpT}
            )
    return in_maps


_PROGRAM = None


def kernel(x, Wk, Wq, Wv, Wp, bp):
    global _PROGRAM
    x = np.asarray(x, np.float32)
    bp = np.asarray(bp, np.float32)
    B, T, _ = x.shape

    if _PROGRAM is None:
        _PROGRAM = build_program(T)
    nc = _PROGRAM

    in_maps = shard_inputs(x, Wk, Wq, Wv, Wp, T=T)
    res = run_bass_kernel_spmd(nc, in_maps, core_ids=list(range(N_CORES)))
    parts = [r["out"] for r in res.results]

    out = np.zeros((B, T, C), np.float32)
    for g in range(N_GROUPS):
        for b in range(B):
            out[b] += parts[g * N_BATCH + b]
    out += bp
    return out
